# revision 1
# baseline (speedup 1.0000x reference)
"""Trainium2 Bass kernel for nn_CryptoNet: 3-layer LSTM + per-step BatchNorm + 2-layer head.

Strategy: 8-way data parallel over batch (128 samples/core), zero cross-core
communication (BN uses per-shard batch stats, which the sharding hint allows;
measured rel err vs full-batch stats: 1.8e-3).

Per-core design:
  - batch (128) on SBUF partitions for gate/cell math; recurrent matmuls
    compute gates[128b, 4H] = act.T-stationary @ W.T-moving in float32r
    (full-rate PE) with the bias added via a K=1 ones-row matmul.
  - h is PE-transposed; BN stats via bn_stats/bn_aggr on h.T
    (features-on-partitions); rstd computed with a DVE-only Newton rsqrt
    (keeps the scalar engine on one activation-table set the whole kernel);
    BN apply fused into one scalar-engine activation with per-partition
    scale/bias -> produces z.T directly = next layer's stationary operand.
  - software pipeline across layers: superstep s runs L1@t=s, L2@t=s-1,
    L3@t=s-2, head@t=s-3 so PE/ACT/DVE/DMA overlap across layers.
  - gate order host-permuted to (i,f,o,g): one Sigmoid covers 3H.
  - softmax over 2 classes = sigmoid of logit difference.
"""

import sys
import numpy as np

for p in ("/opt/trn_rl_repo", "/opt/trn_rl_repo/concourse"):
    if p not in sys.path:
        sys.path.insert(0, p)

B, T, I = 1024, 256, 128
T_STEPS = T  # override for small-scale testing
T_RUN = None  # loop steps; defaults to T_STEPS
H1, H2, H3 = 256, 256, 32
NCORES = 8
BL = B // NCORES  # local batch per core = 128
EPS = 1e-5

_CACHE = {}


def _gate_perm(H):
    # torch gate order (i, f, g, o) -> (i, f, o, g)
    idx = np.arange(4 * H)
    i, f, g, o = np.split(idx, 4)
    return np.concatenate([i, f, o, g])


def _build(dt_w, dt_x, run=None):
    import concourse.bass as bass
    import concourse.tile as tile
    import concourse.mybir as mybir
    from concourse import bacc
    from concourse.masks import make_identity

    f32 = mybir.dt.float32
    u32 = mybir.dt.uint32
    f32r = mybir.dt.float32r
    AF = mybir.ActivationFunctionType
    OP = mybir.AluOpType
    r32 = lambda ap: ap.bitcast(f32r)

    nc = bacc.Bacc("TRN2", target_bir_lowering=False, debug=False,
                   num_devices=NCORES)

    with tile.TileContext(nc) as tc:
        dr = lambda name, shape, dt: nc.dram_tensor(
            name, shape, dt, kind="ExternalInput").ap()
        xT = dr("xT", [I, T_STEPS, BL], dt_x)      # host pre-transposed [i, t, b]
        w1t = dr("w1t", [I, 4 * H1], dt_w)         # Wih1.T, gate-reordered
        wh1t = dr("wh1t", [H1, 4 * H1], mybir.dt.bfloat16)
        w2t = dr("w2t", [H2, 4 * H2], mybir.dt.bfloat16)
        wh2t = dr("wh2t", [H2, 4 * H2], mybir.dt.bfloat16)
        w3t = dr("w3t", [H2, 4 * H3], mybir.dt.bfloat16)
        wh3ta = dr("wh3ta", [H3 + 1, 4 * H3], mybir.dt.bfloat16)  # [Whh3.T ; b3]
        b1r = dr("b1r", [1, 4 * H1], dt_w)
        b2r = dr("b2r", [1, 4 * H2], dt_w)
        gball = dr("gball", [128, 10], f32)  # gamma cols 0:5, beta cols 5:10
        wlt = dr("wlt", [H3, 2], mybir.dt.bfloat16)       # Wl.T
        blp = dr("blp", [2, 1], f32)         # bl as per-partition bias
        wd = dr("wd", [2, 1], mybir.dt.bfloat16)          # Wl2[0]-Wl2[1] as column
        headc = dr("headc", [128, 1], f32)   # bl2[0]-bl2[1] replicated
        y = nc.dram_tensor("y", [BL, 2 * T_STEPS], f32,
                           kind="ExternalOutput").ap()

        with (
            tc.tile_pool(name="const", bufs=1) as const,
            tc.tile_pool(name="state", bufs=1) as state,
            tc.tile_pool(name="xin", bufs=3) as xin,
            tc.tile_pool(name="work", bufs=3) as work,
            tc.tile_pool(name="zt", bufs=3) as ztp,
            tc.tile_pool(name="g1p", bufs=1, space="PSUM") as g1p,
            tc.tile_pool(name="g2p", bufs=1, space="PSUM") as g2p,
            tc.tile_pool(name="smp", bufs=1, space="PSUM") as smp,
            tc.tile_pool(name="tp1", bufs=1, space="PSUM") as tp1,
            tc.tile_pool(name="tp2", bufs=1, space="PSUM") as tp2,
            tc.tile_pool(name="hdp", bufs=1, space="PSUM") as hdp,
        ):
            # ---------------- constants ----------------
            ident = const.tile([128, 128], f32)
            make_identity(nc, ident)
            bf16 = mybir.dt.bfloat16
            ident_b = const.tile([128, 128], bf16)
            make_identity(nc, ident_b)
            ones_row = const.tile([1, 128], dt_w)
            nc.vector.memset(ones_row.bitcast(f32), 1.0)
            magic_t = const.tile([128, 5], u32)
            nc.vector.memset(magic_t, 0x5F3759DF)

            def load(name, shape, dt, src):
                t = const.tile(shape, dt, tag=name)
                nc.sync.dma_start(t[:], src)
                return t

            w1t_s = load("w1t", [128, 4 * H1], dt_w, w1t[:])
            wh1t_s = load("wh1t", [128, 2, 4 * H1], bf16,
                          wh1t.rearrange("(k p) n -> p k n", p=128))
            w2t_s = load("w2t", [128, 2, 4 * H2], bf16,
                         w2t.rearrange("(k p) n -> p k n", p=128))
            wh2t_s = load("wh2t", [128, 2, 4 * H2], bf16,
                          wh2t.rearrange("(k p) n -> p k n", p=128))
            w3t_s = load("w3t", [128, 2, 4 * H3], bf16,
                         w3t.rearrange("(k p) n -> p k n", p=128))
            wh3ta_s = load("wh3ta", [H3 + 1, 4 * H3], bf16, wh3ta[:])
            b1r_s = load("b1r", [1, 4 * H1], dt_w, b1r[:])
            b2r_s = load("b2r", [1, 4 * H2], dt_w, b2r[:])
            gball_s = load("gball", [128, 10], f32, gball[:])
            wlt_s = load("wlt", [H3, 2], bf16, wlt[:])
            blp_s = load("blp", [2, 1], f32, blp[:])
            wd_s = load("wd", [2, 1], bf16, wd[:])
            headc_s = load("headc", [128, 1], f32, headc[:])
            nheadc_s = const.tile([128, 1], f32)
            nc.vector.tensor_scalar_mul(nheadc_s, headc_s, -1.0)

            # ---------------- persistent state ----------------
            c1 = state.tile([128, H1], f32)
            c2 = state.tile([128, H2], f32)
            c3 = state.tile([128, H3], f32)
            h1T = state.tile([128, 2, 128], bf16)   # feat-part, batch-free
            h2T = state.tile([128, 2, 128], bf16)
            h3Ta = state.tile([H3 + 1, 128], f32)  # last row = ones (bias)
            out_sb = state.tile([128, 2 * T_STEPS], f32)
            for tens in (c1, c2, c3):
                nc.vector.memset(tens, 0.0)
            for tens in (h1T, h2T):
                nc.vector.memset(tens, 0.0)
            nc.vector.memset(h3Ta[0:H3, :], 0.0)
            nc.vector.memset(h3Ta[H3:H3 + 1, :], 1.0)

            XCH = 8  # x chunk length (steps per DMA)

            def cell_math(gates, H, c, tag, split=False, dth=None):
                """sigmoid/tanh + cell update; returns h [128, H] (BF).

                split=True issues sigmoid(i,f) on the first PSUM bank so it
                can start before the second bank's matmuls finish."""
                dth = dth or f32
                sig = work.tile([128, 3 * H], dth, tag=f"sig{tag}")
                tg = work.tile([128, H], dth, tag=f"tg{tag}")
                cn = work.tile([128, H], f32, tag=f"cn{tag}")
                tm = work.tile([128, H], dth, tag=f"tm{tag}")
                h = work.tile([128, H], dth, tag=f"h{tag}")
                if split:
                    nc.scalar.activation(sig[:, 0:2 * H], gates[:, 0:2 * H],
                                         AF.Sigmoid)
                else:
                    nc.scalar.activation(sig, gates[:, 0:3 * H], AF.Sigmoid)
                nc.vector.tensor_mul(cn, sig[:, H:2 * H], c)     # f*c first
                nc.scalar.activation(tg, gates[:, 3 * H:4 * H], AF.Tanh)
                if split:
                    nc.scalar.activation(sig[:, 2 * H:3 * H],
                                         gates[:, 2 * H:3 * H], AF.Sigmoid)
                nc.vector.tensor_mul(tm, sig[:, 0:H], tg)        # i*g~
                nc.vector.tensor_add(c, cn, tm)
                nc.scalar.activation(tg, c, AF.Tanh)             # tanh(c)
                nc.vector.tensor_mul(h, sig[:, 2 * H:3 * H], tg)
                return h

            R = run if run is not None else T_STEPS
            NS = R + 6
            mvq = {}

            def get_mv(i):
                if i not in mvq:
                    mvq[i] = work.tile([128, 5, 2], f32, tag="mv", bufs=4,
                                       name="mvt")
                return mvq[i]

            h3q = {}
            h3init = work.tile([H3 + 1, 128], bf16, tag="h3a", bufs=4,
                               name="h3init")
            nc.vector.memset(h3init[0:H3, :], 0.0)
            nc.vector.memset(h3init[H3:H3 + 1, :], 1.0)
            h3q[2] = h3init
            z1T_prev = z2T_prev = z3T_prev = None
            h2T_ps_prev = None

            for s in range(NS):
                st6 = work.tile([128, 5, 6], f32, tag="st6")
                h1T_ps = h2T_ps = None

                # ---------- L1 @ t=s ----------
                if s < R:
                    ti = s % XCH
                    if ti == 0:
                        xT_sb = xin.tile([128, XCH, 128], dt_x, tag="x")
                        nc.sync.dma_start(xT_sb, xT[:, s:s + XCH, :])
                    g1 = g1p.tile([128, 4 * H1], f32, tag="g1")
                    for nj in range(2):
                        nn_ = slice(512 * nj, 512 * (nj + 1))
                        nc.tensor.matmul(g1[:, nn_], ones_row, b1r_s[:, nn_],
                                         start=True, stop=False)
                        nc.tensor.matmul(g1[:, nn_], xT_sb[:, ti, :],
                                         w1t_s[:, nn_], start=False, stop=False)
                        for k in range(2):
                            nc.tensor.matmul(g1[:, nn_], h1T[:, k, :],
                                             wh1t_s[:, k, nn_],
                                             start=False, stop=(k == 1))
                    h1 = cell_math(g1, H1, c1, "a", split=True, dth=bf16)
                    h1T_ps = tp1.tile([128, 2, 128], bf16, tag="tp1")
                    for j in range(2):
                        nc.tensor.transpose(h1T_ps[:, j, :],
                                            h1[:, j * 128:(j + 1) * 128],
                                            ident_b)
                    nc.vector.tensor_copy(h1T[:, :, :], h1T_ps[:, :, :])
                    mvc = get_mv(s)
                    for j in range(2):
                        nc.vector.bn_stats(st6[:, j, :], h1T_ps[:, j, :])
                        nc.vector.bn_aggr(mvc[:, j, :], st6[:, j, :])

                # ---------- L2 @ t=s-1 ----------
                if 1 <= s <= R:
                    z1T = z1T_prev
                    g2 = g2p.tile([128, 4 * H2], f32, tag="g2")
                    for nj in range(2):
                        nn_ = slice(512 * nj, 512 * (nj + 1))
                        nc.tensor.matmul(g2[:, nn_], ones_row, b2r_s[:, nn_],
                                         start=True, stop=False)
                        for k in range(2):
                            nc.tensor.matmul(g2[:, nn_], z1T[:, k, :],
                                             w2t_s[:, k, nn_],
                                             start=False, stop=False)
                        for k in range(2):
                            nc.tensor.matmul(g2[:, nn_], h2T[:, k, :],
                                             wh2t_s[:, k, nn_],
                                             start=False, stop=(k == 1))
                    h2 = cell_math(g2, H2, c2, "b", split=True, dth=bf16)
                    h2T_ps = tp2.tile([128, 2, 128], bf16, tag="tp2")
                    for j in range(2):
                        nc.tensor.transpose(h2T_ps[:, j, :],
                                            h2[:, j * 128:(j + 1) * 128],
                                            ident_b)
                    nc.vector.tensor_copy(h2T[:, :, :], h2T_ps[:, :, :])
                    mvn = get_mv(s + 1)
                    for j in range(2):
                        nc.vector.bn_stats(st6[:, 2 + j, :], h2T_ps[:, j, :])
                        nc.vector.bn_aggr(mvn[:, 2 + j, :], st6[:, 2 + j, :])

                # ---------- L3 @ t=s-3 ----------
                if 3 <= s <= R + 2:
                    z2T = z2T_prev
                    g3 = smp.tile([128, 4 * H3], f32, tag="sm")
                    nc.tensor.matmul(g3, z2T[:, 0, :], w3t_s[:, 0, :],
                                     start=True, stop=False)
                    nc.tensor.matmul(g3, z2T[:, 1, :], w3t_s[:, 1, :],
                                     start=False, stop=False)
                    nc.tensor.matmul(g3, h3q[s - 1], wh3ta_s,
                                     start=False, stop=True)
                    h3 = cell_math(g3, H3, c3, "c", dth=bf16)
                    h3T_ps = smp.tile([H3, 128], bf16, tag="sm")
                    nc.tensor.transpose(h3T_ps, h3[:, 0:H3], ident_b)
                    h3aug = work.tile([H3 + 1, 128], bf16, tag="h3a", bufs=4)
                    nc.vector.tensor_copy(h3aug[0:H3, :], h3T_ps)
                    nc.vector.memset(h3aug[H3:H3 + 1, :], 1.0)
                    mvn2 = get_mv(s + 2)
                    nc.vector.bn_stats(st6[0:H3, 4, :], h3aug[0:H3, :])
                    nc.vector.bn_aggr(mvn2[0:H3, 4, :], st6[0:H3, 4, :])
                    h3q[s] = h3aug

                # ---------- head @ t=s-6 ----------
                if 6 <= s <= R + 5:
                    t_out = s - 6
                    z3T = z3T_prev
                    o1t = smp.tile([2, 128], f32, tag="sm")
                    nc.tensor.matmul(o1t, wlt_s, z3T, start=True, stop=True)
                    relu1 = work.tile([2, 128], bf16, tag="rl")
                    nc.scalar.activation(relu1, o1t, AF.Relu, bias=blp_s)
                    dcol = smp.tile([128, 1], f32, tag="sm")
                    nc.tensor.matmul(dcol, relu1, wd_s, start=True, stop=True)
                    nc.scalar.activation(out_sb[:, 2 * t_out:2 * t_out + 1],
                                         dcol, AF.Sigmoid, bias=headc_s,
                                         scale=1.0)
                    nc.vector.tensor_scalar(
                        out=out_sb[:, 2 * t_out + 1:2 * t_out + 2],
                        in0=out_sb[:, 2 * t_out:2 * t_out + 1],
                        scalar1=-1.0, scalar2=1.0, op0=OP.mult, op1=OP.add)

                if s <= R + 4:
                    # ---- fused Newton rsqrt + coefs (slack tail) ----
                    mv = get_mv(s)
                    cs = slice(0, 5)
                    ve = work.tile([128, 5], f32, tag="ve")
                    t2 = work.tile([128, 5], f32, tag="t2")
                    u2 = work.tile([128, 5], f32, tag="u2")
                    y1 = work.tile([128, 5], f32, tag="y1")
                    s_ = work.tile([128, 5], f32, tag="s_")
                    tt = work.tile([128, 5], f32, tag="tt")
                    vecs = ve[:, cs]
                    nc.vector.tensor_scalar_add(vecs, mv[:, cs, 1], EPS)
                    nc.vector.tensor_scalar(
                        out=t2[:, cs].bitcast(u32), in0=vecs.bitcast(u32),
                        scalar1=1, scalar2=None, op0=OP.arith_shift_right)
                    nc.gpsimd.tensor_sub(y1[:, cs].bitcast(u32),
                                         magic_t[:, cs], t2[:, cs].bitcast(u32))
                    nc.gpsimd.tensor_mul(u2[:, cs], y1[:, cs], y1[:, cs])
                    nc.gpsimd.tensor_mul(t2[:, cs], vecs, u2[:, cs])
                    nc.vector.tensor_scalar(out=u2[:, cs], in0=t2[:, cs],
                                            scalar1=-0.5, scalar2=1.5,
                                            op0=OP.mult, op1=OP.add)
                    nc.gpsimd.tensor_mul(t2[:, cs], y1[:, cs], u2[:, cs])
                    nc.gpsimd.tensor_mul(u2[:, cs], t2[:, cs], t2[:, cs])
                    nc.gpsimd.tensor_mul(y1[:, cs], vecs, u2[:, cs])
                    nc.vector.tensor_scalar(out=u2[:, cs], in0=y1[:, cs],
                                            scalar1=-0.5, scalar2=1.5,
                                            op0=OP.mult, op1=OP.add)
                    nc.gpsimd.tensor_mul(y1[:, cs], t2[:, cs], u2[:, cs])
                    nc.gpsimd.tensor_mul(s_[:, cs], y1[:, cs], gball_s[:, cs])
                    nc.gpsimd.tensor_mul(u2[:, cs], mv[:, cs, 0], s_[:, cs])
                    nc.gpsimd.tensor_sub(tt[:, cs], gball_s[:, 5:10], u2[:, cs])

                    # ---- BN applies on the h-states saved last superstep ----
                if s < R:
                    z1T = ztp.tile([128, 2, 128], bf16, tag="z1")
                    for j in range(2):
                        nc.vector.tensor_scalar(
                            out=z1T[:, j, :], in0=h1T[:, j, :],
                            scalar1=s_[:, j:j + 1], scalar2=tt[:, j:j + 1],
                            op0=OP.mult, op1=OP.add)
                    z1T_prev = z1T
                if 2 <= s <= R + 1:
                    z2T = ztp.tile([128, 2, 128], bf16, tag="z2")
                    for j in range(2):
                        nc.vector.tensor_scalar(
                            out=z2T[:, j, :], in0=h2T_ps_prev[:, j, :],
                            scalar1=s_[:, 2 + j:3 + j],
                            scalar2=tt[:, 2 + j:3 + j],
                            op0=OP.mult, op1=OP.add)
                    z2T_prev = z2T
                if 5 <= s <= R + 4:
                    z3T = ztp.tile([H3, 128], bf16, tag="z3")
                    nc.vector.tensor_scalar(
                        out=z3T, in0=h3q[s - 2][0:H3, :],
                        scalar1=s_[0:H3, 4:5], scalar2=tt[0:H3, 4:5],
                        op0=OP.mult, op1=OP.add)
                    z3T_prev = z3T
                h2T_ps_prev = h2T_ps
                if s - 3 in h3q:
                    del h3q[s - 3]


            nc.sync.dma_start(y, out_sb)

    nc.compile()
    return nc


def _prep_host(inputs, np_w, np_x):
    gp1 = _gate_perm(H1)
    gp2 = _gate_perm(H2)
    gp3 = _gate_perm(H3)
    f = lambda a: np.ascontiguousarray(a, dtype=np.float32)

    import ml_dtypes
    bf = ml_dtypes.bfloat16
    w1t = f(inputs["Wih1"][gp1].T).astype(np_w)
    wh1t = f(inputs["Whh1"][gp1].T).astype(bf)
    w2t = f(inputs["Wih2"][gp2].T).astype(bf)
    wh2t = f(inputs["Whh2"][gp2].T).astype(bf)
    w3t = f(inputs["Wih3"][gp3].T).astype(bf)
    wh3t = f(inputs["Whh3"][gp3].T).astype(bf)
    b1 = f(inputs["bih1"] + inputs["bhh1"])[gp1][None, :]
    b2 = f(inputs["bih2"] + inputs["bhh2"])[gp2][None, :]
    b3 = f(inputs["bih3"] + inputs["bhh3"])[gp3][None, :]
    wh3ta = np.concatenate([wh3t, b3.astype(bf)], axis=0)

    def cols128(v):  # [256] -> [128, 2]
        return np.ascontiguousarray(v.reshape(2, 128).T, dtype=np.float32)

    gball = np.zeros((128, 10), np.float32)
    gball[:, 0:2] = cols128(f(inputs["g1"]))
    gball[:, 2:4] = cols128(f(inputs["g2"]))
    gball[0:H3, 4] = f(inputs["g3"])
    gball[:, 5:7] = cols128(f(inputs["b1"]))
    gball[:, 7:9] = cols128(f(inputs["b2"]))
    gball[0:H3, 9] = f(inputs["b3"])

    wlt = f(inputs["Wl"].T).astype(bf)
    blp = f(inputs["bl"])[:, None]
    wd = f(inputs["Wl2"][0] - inputs["Wl2"][1])[:, None].astype(bf)
    dc = float(inputs["bl2"][0] - inputs["bl2"][1])
    headc = np.full((128, 1), dc, np.float32)

    shared = dict(w1t=w1t, wh1t=wh1t, w2t=w2t, wh2t=wh2t, w3t=w3t,
                  wh3ta=wh3ta, b1r=b1, b2r=b2, gball=gball,
                  wlt=wlt, blp=blp, wd=wd, headc=headc)

    x = np.asarray(inputs["x"], dtype=np.float32)
    in_maps = []
    for c in range(NCORES):
        xc = x[c * BL:(c + 1) * BL]
        xTc = np.ascontiguousarray(
            xc[:, :T_STEPS, :].transpose(2, 1, 0)).astype(np_x)
        m = dict(shared)
        m["xT"] = xTc
        in_maps.append(m)
    return in_maps


def kernel(**inputs):
    import concourse.mybir as mybir
    from concourse import bass_utils

    dt_w = mybir.dt.float32r
    dt_x = mybir.dt.float32r
    np_w = np.float32
    np_x = np.float32

    key = ("v3", str(dt_w), str(dt_x), T_STEPS, T_RUN)
    if key not in _CACHE:
        _CACHE[key] = _build(dt_w, dt_x, run=T_RUN)
    nc = _CACHE[key]

    in_maps = _prep_host(inputs, np_w, np_x)
    res = bass_utils.run_bass_kernel_spmd(nc, in_maps,
                                          core_ids=list(range(NCORES)))
    out = np.empty((B, T_STEPS, 2), np.float32)
    for c in range(NCORES):
        out[c * BL:(c + 1) * BL] = res.results[c]["y"].reshape(BL, T_STEPS, 2)
    return out

